# revision 8
# baseline (speedup 1.0000x reference)
"""Trainium2 Bass kernel for ApproxLTCLayer (8-core data-parallel over batch).

Reference computation (per batch b, with t == b the "time" scalar):
    x = inputs[b].reshape(T=4096, D=16)
    z = sigma[u,d] * (x[t,d] - mu[u,d])
    out[t,u] = sum_d [ (x0[u]-A[u,d]) * exp(-(omega+sigmoid(z))*b) * sigmoid(-z) ]
               + sum_d A[u,d]

Key observation: per (u,d,b) the summand is a smooth univariate function of
x[t,d].  Instead of evaluating tanh+exp per (t,u,d) element (16 full ACT
passes — the original bottleneck), approximate ALL 64*16 per-(u,d) functions
in a shared tanh ridge basis of J=8 neurons:
    F_{u,d}(x) ~= sum_j C[u,d,j] * tanh(s*x + b_j)
so out[t,u] = sum_{d,j} C * tau_j(x[t,d]) + base[u], i.e. ONE 128-deep PE
contraction.  C is a per-(u,d) ridge least-squares fit against the exact
function on a Gauss-weighted grid (host, trivial cost).  rel err ~4.2e-3
(gate 2e-2), dominated by the basis fit, not quantization.

Device layout (per core): partitions p = (r, d) with d = p%16, r = p//16;
neuron j = r.  xbc host-pre-broadcast to [128, 4096] fp16, DRAM-packed
chunk-contiguous (256KB contiguous per chunk).
  ACT: tau = tanh(s*x + b_p)     bf16, per-partition bias AP, scale imm —
       the ONLY transcendental work: 1 pass x 4096 cols (was 16 passes).
  PE : psum = cmat.T @ tau  (cmat stationary [128,64] bf16, tau MOVING 512
       cols/matmul = one PSUM bank, start+stop in one shot).  Consecutive
       512-col output blocks land in the SAME bank at partition offsets 0/64,
       so DVE evacuation runs full-width [128,512] (half the evac time) and
       one DMA per bank writes both t-halves via a rearranged DRAM AP.
Out DRAM is [64, 4096] fp16 (contiguous per partition); host transposes and
adds base[u].  DMA ordering tuned to the ~3us queue-ramp/semaphore latency:
chunk0, chunk1 and the tiny bias tensor ramp three queues in parallel; the
last ACT pieces are 512 wide so the final matmul->evac->DMA chain is short.
"""

import contextlib
import ctypes
import os
import sys
import types

import numpy as np

from concourse import bacc, bass, mybir, tile
from concourse.bass_utils import run_bass_kernel_spmd


def _ensure_axon_hooks_module():
    """bass_utils imports antenv.axon_hooks for NTFF profiling under axon;
    this image's antenv lacks it.  Provide a shim wired to libaxon_pjrt.so."""
    try:
        import antenv.axon_hooks  # noqa: F401

        return
    except ImportError:
        pass

    mod = types.ModuleType("antenv.axon_hooks")
    state = {"hook": None}

    def set_axon_ntff_profile_hook(h):
        state["hook"] = h

    def get_axon_ntff_profile_hook():
        return state["hook"]

    mod.set_axon_ntff_profile_hook = set_axon_ntff_profile_hook
    mod.get_axon_ntff_profile_hook = get_axon_ntff_profile_hook
    sys.modules["antenv.axon_hooks"] = mod
    import antenv

    antenv.axon_hooks = mod

    so_path = "/opt/axon/libaxon_pjrt.so"
    if not os.path.exists(so_path):
        return
    try:
        lib = ctypes.CDLL(so_path)
    except OSError:
        return
    if not hasattr(lib, "axon_start_nrt_profile"):
        return
    lib.axon_start_nrt_profile.argtypes = [
        ctypes.POINTER(ctypes.c_int64),
        ctypes.c_size_t,
    ]
    lib.axon_start_nrt_profile.restype = ctypes.c_int64
    lib.axon_stop_nrt_profile.argtypes = [ctypes.c_char_p]
    lib.axon_stop_nrt_profile.restype = ctypes.c_int64

    @contextlib.contextmanager
    def _hook(output_dir, device_ids):
        import jax

        jax.devices()
        if device_ids:
            ids = (ctypes.c_int64 * len(device_ids))(*device_ids)
            rc = lib.axon_start_nrt_profile(ids, len(device_ids))
        else:
            rc = lib.axon_start_nrt_profile(None, 0)
        if rc != 0:
            raise RuntimeError(f"axon_start_nrt_profile rc={rc}")
        try:
            yield
        finally:
            n = lib.axon_stop_nrt_profile(str(output_dir).encode())
            print(f"profile: {n} file(s) written to {output_dir}", file=sys.stderr)

    set_axon_ntff_profile_hook(_hook)


_ensure_axon_hooks_module()

OMEGA = 0.1
B, T, D, U = 8, 4096, 16, 64
J = 8            # tanh basis size (J//8 ACT passes)
NCH = 4          # xbc DMA chunks (chunk-contiguous DRAM layout)
CH = T // NCH
NCORES = 8
F32 = mybir.dt.float32
BF16 = mybir.dt.bfloat16
FP16 = mybir.dt.float16

# ridge-fit hyperparameters (validated off-line: rel err ~4.2e-3 at J=8)
FIT_GMAX = 5.6
FIT_GPTS = 2001
FIT_LAM = 1e-3
FIT_WFLOOR = 3e-4

_cached_nc = None
last_result = None


def _basis():
    """Uniform tanh grid over the active x range: tanh(s*x + b_j), s shared."""
    c = np.linspace(-4.2, 4.2, J)
    s = np.full(J, 1.0 / (c[1] - c[0]))
    return s, -s * c


def _build_program():
    nc = bacc.Bacc("TRN2", target_bir_lowering=False, debug=False, num_devices=NCORES)

    s, _ = _basis()
    scale_imm = float(s[0])

    # xbc packed chunk-contiguous: DRAM row 128*ci + p holds
    # x[1024*ci : 1024*(ci+1), d(p)] — each chunk is one contiguous 256KB read.
    xbc_d = nc.declare_dram_parameter("xbc", [NCH * 128, CH], FP16, isOutput=False)
    # per-partition tanh bias (8B/partition — tiny, lands first)
    pb_d = nc.declare_dram_parameter("pb", [128, 2], F32, isOutput=False)
    cmat_d = nc.declare_dram_parameter("cmat", [128, U], BF16, isOutput=False)
    # transposed output: out[u, t] fp16; host transposes back and adds base.
    out = nc.declare_dram_parameter("out", [U, T], FP16, isOutput=True)

    out_ap = out.ap()

    with tile.TileContext(nc) as tc:
        with (
            tc.tile_pool(name="const", bufs=1) as cpool,
            tc.tile_pool(name="xb", bufs=1) as xpool,
            tc.tile_pool(name="work", bufs=3) as wpool,
            tc.tile_pool(name="psum", bufs=1, space="PSUM") as ppool,
        ):
            # Three queues ramp their first DMA in parallel (~3us fixed
            # issue->semaphore latency each): chunk0 on sync, chunk1 on
            # gpsimd, bias on the scalar queue (issued before the table load
            # — ACT is gated on chunk0 until ~10us anyway).
            xbc = xpool.tile([128, T], FP16, tag="xbc")
            pb_sb = cpool.tile([128, 2], F32, tag="pb")
            nc.scalar.dma_start(out=pb_sb[:], in_=pb_d.ap()[:])

            # Warm the ACT table set so the ~2.7us PSEUDO_LOAD_ACT_FUNC_SET
            # overlaps the input DMAs instead of gating the first real TANH.
            dum = cpool.tile([1, 2], F32, tag="dum")
            nc.gpsimd.memset(dum[:], 0.0)
            dum2 = cpool.tile([1, 2], F32, tag="dum2")
            nc.scalar.activation(dum2[:], dum[:], mybir.ActivationFunctionType.Tanh)

            def chunk_dma(eng, ci):
                eng.dma_start(
                    out=xbc[:, ci * CH : (ci + 1) * CH],
                    in_=xbc_d.ap()[128 * ci : 128 * (ci + 1), :],
                )

            chunk_dma(nc.sync, 0)
            chunk_dma(nc.gpsimd, 1)
            cm_sb = cpool.tile([128, U], BF16, tag="cm")
            nc.gpsimd.dma_start(out=cm_sb[:], in_=cmat_d.ap()[:])
            chunk_dma(nc.sync, 2)
            chunk_dma(nc.gpsimd, 3)

            # psum: block k (t cols [512k, 512k+512)) lives in bank k//2 at
            # partition offset 64*(k%2) — two blocks share a bank so the DVE
            # evac below runs on all 128 partitions.
            ps = ppool.tile([128, T // 2], F32, tag="ps", name="ps")

            # ACT pieces: chunk-aligned head, 512-wide tail for a short drain
            pieces = [(0, 1024), (1024, 1024), (2048, 1024), (3072, 512), (3584, 512)]
            for c0, w in pieces:
                tau = wpool.tile([128, w], BF16, tag="tau")
                nc.scalar.activation(
                    tau[:],
                    xbc[:, c0 : c0 + w],
                    mybir.ActivationFunctionType.Tanh,
                    bias=pb_sb[:, 0:1],
                    scale=scale_imm,
                )
                for sl in range(w // 512):
                    k = c0 // 512 + sl                     # global 512-block
                    bk, half = k // 2, k % 2
                    # one matmul output == one PSUM bank half; single pass,
                    # each matmul opens and closes its accumulation group.
                    nc.tensor.matmul(
                        ps[64 * half : 64 * half + 64, 512 * bk : 512 * bk + 512],
                        lhsT=cm_sb[:],
                        rhs=tau[:, 512 * sl : 512 * (sl + 1)],
                        start=True,
                        stop=True,
                    )
                    if half == 1:
                        # both halves of bank bk are final: evacuate full-width
                        ev = wpool.tile([128, 512], FP16, tag="ev", bufs=4, name="ev")
                        nc.vector.tensor_scalar_mul(
                            ev[:], ps[:, 512 * bk : 512 * bk + 512], 1.0
                        )
                        # two DMAs: partition halves map to adjacent t-ranges
                        eng = nc.sync if bk % 2 == 0 else nc.gpsimd
                        eng2 = nc.gpsimd if bk % 2 == 0 else nc.sync
                        eng.dma_start(
                            out=out_ap[:, 1024 * bk : 1024 * bk + 512],
                            in_=ev[0:64, :],
                        )
                        eng2.dma_start(
                            out=out_ap[:, 1024 * bk + 512 : 1024 * (bk + 1)],
                            in_=ev[64:128, :],
                        )

    nc.compile()
    return nc


def _host_prep(inputs, A, sigma, mu, x0):
    """Build the 8 per-core input maps (fit C on host, package tensors)."""
    import ml_dtypes

    inputs = np.ascontiguousarray(inputs, dtype=np.float32)
    A = np.asarray(A, dtype=np.float64)
    sigma = np.asarray(sigma, dtype=np.float64)
    mu = np.asarray(mu, dtype=np.float64)
    x0 = np.asarray(x0, dtype=np.float64)

    s, bb = _basis()

    # ---- ridge fit of all per-(u,d) target functions in the shared basis ----
    xg = np.linspace(-FIT_GMAX, FIT_GMAX, FIT_GPTS)
    wt = np.sqrt(np.exp(-0.5 * xg**2) + FIT_WFLOOR)
    Phi = np.tanh(s[None, :] * xg[:, None] + bb[None, :])          # [G, J]
    Pw = Phi * wt[:, None]
    Gram = Pw.T @ Pw + FIT_LAM * np.eye(J)
    Gch = np.linalg.cholesky(Gram)

    z = sigma[:, :, None] * (xg[None, None, :] - mu[:, :, None])   # [U,D,G]
    sig_pos = 1.0 / (1.0 + np.exp(-z))
    sig_neg = 1.0 - sig_pos
    coeff0 = x0[:, None] - A                                       # [U,D]

    p = np.arange(128)
    d_idx = p % D
    r_idx = p // D

    pb = np.empty((128, 2), np.float32)
    pb[:, 0] = bb[r_idx]
    pb[:, 1] = 0.0

    in_maps = []
    for b in range(B):
        g = sig_neg * np.exp(-b * sig_pos)                         # [U,D,G]
        F = (coeff0 * np.exp(-OMEGA * b))[:, :, None] * g
        rhs = np.einsum("gj,udg->udj", Pw, F * wt[None, None, :])
        Cb = np.linalg.solve(
            Gch.T, np.linalg.solve(Gch, rhs.reshape(-1, J).T)
        ).T.reshape(U, D, J)                                       # [U,D,J]

        # cmat[p, u] = C[u, d(p), r(p)]
        cmat = np.ascontiguousarray(Cb[:, d_idx, r_idx].T).astype(ml_dtypes.bfloat16)

        xTb = inputs[b].reshape(T, D).T                            # [16, 4096]
        xb128 = np.ascontiguousarray(xTb[d_idx, :]).astype(np.float16)
        # chunk-contiguous packing: [NCH*128, CH]
        xbc = np.ascontiguousarray(
            xb128.reshape(128, NCH, CH).transpose(1, 0, 2).reshape(NCH * 128, CH)
        )
        in_maps.append({"xbc": xbc, "pb": pb, "cmat": cmat})
    return in_maps


def kernel(inputs, A, sigma, mu, x0):
    global _cached_nc, last_result
    if _cached_nc is None:
        _cached_nc = _build_program()
    nc = _cached_nc

    in_maps = _host_prep(inputs, A, sigma, mu, x0)
    base = np.asarray(A, dtype=np.float64).sum(axis=1).astype(np.float32)  # [U]
    trace = os.environ.get("KERNEL_TRACE", "0") == "1"
    res = run_bass_kernel_spmd(nc, in_maps, core_ids=list(range(NCORES)), trace=trace)
    last_result = res
    outs = []
    for c in range(NCORES):
        packed = np.asarray(res.results[c]["out"]).astype(np.float32)  # [U, T]
        outs.append(packed.T + base[None, :])
    return np.stack(outs, axis=0).astype(np.float32)


# revision 10
# speedup vs baseline: 1.0400x; 1.0400x over previous
"""Trainium2 Bass kernel for ApproxLTCLayer (8-core data-parallel over batch).

Reference computation (per batch b, with t == b the "time" scalar):
    x = inputs[b].reshape(T=4096, D=16)
    z = sigma[u,d] * (x[t,d] - mu[u,d])
    out[t,u] = sum_d [ (x0[u]-A[u,d]) * exp(-(omega+sigmoid(z))*b) * sigmoid(-z) ]
               + sum_d A[u,d]

Key observation: per (u,d,b) the summand is a smooth univariate function of
x[t,d].  Instead of evaluating tanh+exp per (t,u,d) element (16 full ACT
passes — the original bottleneck), approximate ALL 64*16 per-(u,d) functions
in a shared tanh ridge basis of J=8 neurons:
    F_{u,d}(x) ~= sum_j C[u,d,j] * tanh(s*x + b_j)
so out[t,u] = sum_{d,j} C * tau_j(x[t,d]) + base[u], i.e. ONE 128-deep PE
contraction.  C is a per-(u,d) ridge least-squares fit against the exact
function on a Gauss-weighted grid (host, trivial cost).  rel err ~4.2e-3
(gate 2e-2), dominated by the basis fit, not quantization.

Device layout (per core): partitions p = (r, d) with d = p%16, r = p//16;
neuron j = r.  xbc host-pre-broadcast to [128, 4096] fp16, DRAM-packed
chunk-contiguous (256KB contiguous per chunk).
  ACT: tau = tanh(s*x + b_p)     bf16, per-partition bias AP, scale imm —
       the ONLY transcendental work: 1 pass x 4096 cols (was 16 passes).
  PE : psum = cmat.T @ tau  (cmat stationary [128,64] bf16, tau MOVING 512
       cols/matmul = one PSUM bank, start+stop in one shot).  Consecutive
       512-col output blocks land in the SAME bank at partition offsets 0/64,
       so DVE evacuation runs full-width [128,512] (half the evac time) and
       one DMA per bank writes both t-halves via a rearranged DRAM AP.
Out DRAM is [64, 4096] fp16 (contiguous per partition); host transposes and
adds base[u].  DMA ordering tuned to the ~3us queue-ramp/semaphore latency:
chunk0, chunk1 and the tiny bias tensor ramp three queues in parallel; the
last ACT pieces are 512 wide so the final matmul->evac->DMA chain is short.
"""

import contextlib
import ctypes
import os
import sys
import types

import numpy as np

from concourse import bacc, bass, mybir, tile
from concourse.bass_utils import run_bass_kernel_spmd


def _ensure_axon_hooks_module():
    """bass_utils imports antenv.axon_hooks for NTFF profiling under axon;
    this image's antenv lacks it.  Provide a shim wired to libaxon_pjrt.so."""
    try:
        import antenv.axon_hooks  # noqa: F401

        return
    except ImportError:
        pass

    mod = types.ModuleType("antenv.axon_hooks")
    state = {"hook": None}

    def set_axon_ntff_profile_hook(h):
        state["hook"] = h

    def get_axon_ntff_profile_hook():
        return state["hook"]

    mod.set_axon_ntff_profile_hook = set_axon_ntff_profile_hook
    mod.get_axon_ntff_profile_hook = get_axon_ntff_profile_hook
    sys.modules["antenv.axon_hooks"] = mod
    import antenv

    antenv.axon_hooks = mod

    so_path = "/opt/axon/libaxon_pjrt.so"
    if not os.path.exists(so_path):
        return
    try:
        lib = ctypes.CDLL(so_path)
    except OSError:
        return
    if not hasattr(lib, "axon_start_nrt_profile"):
        return
    lib.axon_start_nrt_profile.argtypes = [
        ctypes.POINTER(ctypes.c_int64),
        ctypes.c_size_t,
    ]
    lib.axon_start_nrt_profile.restype = ctypes.c_int64
    lib.axon_stop_nrt_profile.argtypes = [ctypes.c_char_p]
    lib.axon_stop_nrt_profile.restype = ctypes.c_int64

    @contextlib.contextmanager
    def _hook(output_dir, device_ids):
        import jax

        jax.devices()
        if device_ids:
            ids = (ctypes.c_int64 * len(device_ids))(*device_ids)
            rc = lib.axon_start_nrt_profile(ids, len(device_ids))
        else:
            rc = lib.axon_start_nrt_profile(None, 0)
        if rc != 0:
            raise RuntimeError(f"axon_start_nrt_profile rc={rc}")
        try:
            yield
        finally:
            n = lib.axon_stop_nrt_profile(str(output_dir).encode())
            print(f"profile: {n} file(s) written to {output_dir}", file=sys.stderr)

    set_axon_ntff_profile_hook(_hook)


_ensure_axon_hooks_module()

OMEGA = 0.1
B, T, D, U = 8, 4096, 16, 64
J = 8            # tanh basis size (J//8 ACT passes)
NCH = 4          # xbc DMA chunks (chunk-contiguous DRAM layout)
CH = T // NCH
NCORES = 8
F32 = mybir.dt.float32
BF16 = mybir.dt.bfloat16
FP16 = mybir.dt.float16

# ridge-fit hyperparameters (validated off-line: rel err ~4.2e-3 at J=8)
FIT_GMAX = 5.6
FIT_GPTS = 2001
FIT_LAM = 1e-3
FIT_WFLOOR = 3e-4

_cached_nc = None
last_result = None


def _basis():
    """Uniform tanh grid over the active x range: tanh(s*x + b_j), s shared."""
    c = np.linspace(-4.2, 4.2, J)
    s = np.full(J, 1.0 / (c[1] - c[0]))
    return s, -s * c


def _build_program():
    nc = bacc.Bacc("TRN2", target_bir_lowering=False, debug=False, num_devices=NCORES)

    s, _ = _basis()
    scale_imm = float(s[0])

    # xbc packed chunk-contiguous: DRAM row 128*ci + p holds
    # x[1024*ci : 1024*(ci+1), d(p)] — each chunk is one contiguous 256KB read.
    xbc_d = nc.declare_dram_parameter("xbc", [NCH * 128, CH], FP16, isOutput=False)
    # per-partition tanh bias (8B/partition — tiny, lands first)
    pb_d = nc.declare_dram_parameter("pb", [128, 2], F32, isOutput=False)
    cmat_d = nc.declare_dram_parameter("cmat", [128, U], BF16, isOutput=False)
    # transposed output: out[u, t] fp16; host transposes back and adds base.
    out = nc.declare_dram_parameter("out", [U, T], FP16, isOutput=True)

    out_ap = out.ap()

    with tile.TileContext(nc) as tc:
        with (
            tc.tile_pool(name="const", bufs=1) as cpool,
            tc.tile_pool(name="xb", bufs=1) as xpool,
            tc.tile_pool(name="work", bufs=3) as wpool,
            tc.tile_pool(name="psum", bufs=1, space="PSUM") as ppool,
        ):
            # Three queues ramp their first DMA in parallel (~3us fixed
            # issue->semaphore latency each): chunk0 on sync, chunk1 on
            # gpsimd, bias on the scalar queue (issued before the table load
            # — ACT is gated on chunk0 until ~10us anyway).
            xbc = xpool.tile([128, T], FP16, tag="xbc")
            pb_sb = cpool.tile([128, 2], F32, tag="pb")
            nc.scalar.dma_start(out=pb_sb[:], in_=pb_d.ap()[:])

            # Warm the ACT table set so the ~2.7us PSEUDO_LOAD_ACT_FUNC_SET
            # overlaps the input DMAs instead of gating the first real TANH.
            dum = cpool.tile([1, 2], F32, tag="dum")
            nc.gpsimd.memset(dum[:], 0.0)
            dum2 = cpool.tile([1, 2], F32, tag="dum2")
            nc.scalar.activation(dum2[:], dum[:], mybir.ActivationFunctionType.Tanh)

            def chunk_dma(eng, ci):
                eng.dma_start(
                    out=xbc[:, ci * CH : (ci + 1) * CH],
                    in_=xbc_d.ap()[128 * ci : 128 * (ci + 1), :],
                )

            chunk_dma(nc.sync, 0)
            cm_sb = cpool.tile([128, U], BF16, tag="cm")
            nc.gpsimd.dma_start(out=cm_sb[:], in_=cmat_d.ap()[:])
            chunk_dma(nc.gpsimd, 1)
            chunk_dma(nc.sync, 2)
            chunk_dma(nc.gpsimd, 3)

            # psum: block k (t cols [512k, 512k+512)) lives in bank k//2 at
            # partition offset 64*(k%2) — two blocks share a bank so the DVE
            # evac below runs on all 128 partitions.
            ps = ppool.tile([128, T // 2], F32, tag="ps", name="ps")

            # ACT pieces: chunk-aligned head, 512-wide tail for a short drain
            pieces = [(0, 1024), (1024, 1024), (2048, 1024), (3072, 512), (3584, 512)]
            for c0, w in pieces:
                tau = wpool.tile([128, w], BF16, tag="tau")
                nc.scalar.activation(
                    tau[:],
                    xbc[:, c0 : c0 + w],
                    mybir.ActivationFunctionType.Tanh,
                    bias=pb_sb[:, 0:1],
                    scale=scale_imm,
                )
                for sl in range(w // 512):
                    k = c0 // 512 + sl                     # global 512-block
                    bk, half = k // 2, k % 2
                    # one matmul output == one PSUM bank half; single pass,
                    # each matmul opens and closes its accumulation group.
                    nc.tensor.matmul(
                        ps[64 * half : 64 * half + 64, 512 * bk : 512 * bk + 512],
                        lhsT=cm_sb[:],
                        rhs=tau[:, 512 * sl : 512 * (sl + 1)],
                        start=True,
                        stop=True,
                    )
                    if half == 1:
                        # both halves of bank bk are final: evacuate full-width
                        ev = wpool.tile([128, 512], FP16, tag="ev", bufs=4, name="ev")
                        nc.vector.tensor_scalar_mul(
                            ev[:], ps[:, 512 * bk : 512 * bk + 512], 1.0
                        )
                        # two DMAs: partition halves map to adjacent t-ranges.
                        # The final pair goes out on the scalar+sync queues
                        # (both idle by then) so the gpsimd end-drain is short.
                        if bk < 3:
                            eng = nc.sync if bk % 2 == 0 else nc.gpsimd
                            eng2 = nc.gpsimd if bk % 2 == 0 else nc.sync
                        else:
                            eng, eng2 = nc.scalar, nc.sync
                        eng.dma_start(
                            out=out_ap[:, 1024 * bk : 1024 * bk + 512],
                            in_=ev[0:64, :],
                        )
                        eng2.dma_start(
                            out=out_ap[:, 1024 * bk + 512 : 1024 * (bk + 1)],
                            in_=ev[64:128, :],
                        )

    nc.compile()
    return nc


def _host_prep(inputs, A, sigma, mu, x0):
    """Build the 8 per-core input maps (fit C on host, package tensors)."""
    import ml_dtypes

    inputs = np.ascontiguousarray(inputs, dtype=np.float32)
    A = np.asarray(A, dtype=np.float64)
    sigma = np.asarray(sigma, dtype=np.float64)
    mu = np.asarray(mu, dtype=np.float64)
    x0 = np.asarray(x0, dtype=np.float64)

    s, bb = _basis()

    # ---- ridge fit of all per-(u,d) target functions in the shared basis ----
    xg = np.linspace(-FIT_GMAX, FIT_GMAX, FIT_GPTS)
    wt = np.sqrt(np.exp(-0.5 * xg**2) + FIT_WFLOOR)
    Phi = np.tanh(s[None, :] * xg[:, None] + bb[None, :])          # [G, J]
    Pw = Phi * wt[:, None]
    Gram = Pw.T @ Pw + FIT_LAM * np.eye(J)
    Gch = np.linalg.cholesky(Gram)

    z = sigma[:, :, None] * (xg[None, None, :] - mu[:, :, None])   # [U,D,G]
    sig_pos = 1.0 / (1.0 + np.exp(-z))
    sig_neg = 1.0 - sig_pos
    coeff0 = x0[:, None] - A                                       # [U,D]

    p = np.arange(128)
    d_idx = p % D
    r_idx = p // D

    pb = np.empty((128, 2), np.float32)
    pb[:, 0] = bb[r_idx]
    pb[:, 1] = 0.0

    in_maps = []
    for b in range(B):
        g = sig_neg * np.exp(-b * sig_pos)                         # [U,D,G]
        F = (coeff0 * np.exp(-OMEGA * b))[:, :, None] * g
        rhs = np.einsum("gj,udg->udj", Pw, F * wt[None, None, :])
        Cb = np.linalg.solve(
            Gch.T, np.linalg.solve(Gch, rhs.reshape(-1, J).T)
        ).T.reshape(U, D, J)                                       # [U,D,J]

        # cmat[p, u] = C[u, d(p), r(p)]
        cmat = np.ascontiguousarray(Cb[:, d_idx, r_idx].T).astype(ml_dtypes.bfloat16)

        xTb = inputs[b].reshape(T, D).T                            # [16, 4096]
        xb128 = np.ascontiguousarray(xTb[d_idx, :]).astype(np.float16)
        # chunk-contiguous packing: [NCH*128, CH]
        xbc = np.ascontiguousarray(
            xb128.reshape(128, NCH, CH).transpose(1, 0, 2).reshape(NCH * 128, CH)
        )
        in_maps.append({"xbc": xbc, "pb": pb, "cmat": cmat})
    return in_maps


def kernel(inputs, A, sigma, mu, x0):
    global _cached_nc, last_result
    if _cached_nc is None:
        _cached_nc = _build_program()
    nc = _cached_nc

    in_maps = _host_prep(inputs, A, sigma, mu, x0)
    base = np.asarray(A, dtype=np.float64).sum(axis=1).astype(np.float32)  # [U]
    trace = os.environ.get("KERNEL_TRACE", "0") == "1"
    res = run_bass_kernel_spmd(nc, in_maps, core_ids=list(range(NCORES)), trace=trace)
    last_result = res
    outs = []
    for c in range(NCORES):
        packed = np.asarray(res.results[c]["out"]).astype(np.float32)  # [U, T]
        outs.append(packed.T + base[None, :])
    return np.stack(outs, axis=0).astype(np.float32)
